# revision 21
# baseline (speedup 1.0000x reference)
"""Trainium2 Bass kernel for a 3-layer GATv2 encoder (nn_Encoder_14620068675922).

Math notes (matches reference exactly):
  - gate = softmax over a size-1 axis == 1.0 exactly, so the final output z is
    just the GNN branch; the spatial MLP / MHA branch is dead code.
  - GAT biases (bias1..3) and projection biases shift every row equally or feed
    BatchNorm, which is shift-invariant -> biases1..3 dropped; proj biases kept.
  - LeakyReLU(x, s) = (1-s)*relu(x) + s*x. For the attention logits the linear
    term decomposes per node: att . (xl[src]+xr[dst]) = sl[src] + sr[dst], with
    sl = h @ (wlT @ attW), computed during the projection matmuls. Only the
    relu part needs the per-edge feature vectors.

Distribution (8 cores): edges partitioned by destination node (2500 dst nodes
per core, contiguous); weights replicated; per layer one AllGather of the
projected source features xl (node-major, gather-ready) and one tiny AllReduce
of BatchNorm statistics. Per-edge work is done in windows of 128 destination
nodes; segment-softmax via per-head masked matmuls with a destination-match
mask built on the vector engine.
"""

import os
import sys

import numpy as np

for _p in ("/opt/trn_rl_repo",):
    if _p not in sys.path and os.path.isdir(_p):
        sys.path.insert(0, _p)

N = 20000
E = 320000
IN = 512
H = 4
C = 128
HC = H * C
NCORES = 8
ATT_NEG = 0.2
NEG = 0.01
G_CHUNKS = 4  # chunks (of 128 edges) per dma_gather call
EX = 64  # row extension block (lin logit terms + pad, dma_gather needs 256B multiples)


# ---------------------------------------------------------------------------
# host-side graph preprocessing
# ---------------------------------------------------------------------------

def preprocess(edge_index, n, ncores):
    e = edge_index.shape[1]
    src = np.concatenate([edge_index[0], np.arange(n)]).astype(np.int64)
    dst = np.concatenate([edge_index[1], np.arange(n)]).astype(np.int64)
    etot = e + n
    perm = np.argsort(dst, kind="stable")
    ssrc, sdst = src[perm], dst[perm]

    nloc = n // ncores
    nw = (nloc + 127) // 128
    core_id = sdst // nloc
    dl = sdst % nloc          # local dst id
    wloc = dl // 128          # window within core

    counts = np.zeros((ncores, nw), np.int64)
    np.add.at(counts, (core_id, wloc), 1)
    kw = np.maximum(1, (counts.max(axis=0) + 127) // 128)  # chunks per window
    nchunk = int(kw.sum())
    nslot = nchunk * 128
    goff = np.zeros(nw, np.int64)
    goff[1:] = np.cumsum(kw)[:-1]

    gixl_u = np.zeros((ncores, nslot), np.int64)
    gixr_u = np.zeros((ncores, nslot), np.int64)
    q_u = np.full((ncores, nslot), 1000.0, np.float32)
    slot_of_edge = np.empty(etot, np.int64)

    # edges sorted by dst -> (core, window) groups are contiguous
    grp_key = core_id * nw + wloc
    grp_start = np.zeros(ncores * nw + 1, np.int64)
    np.add.at(grp_start, grp_key + 1, 1)
    grp_start = np.cumsum(grp_start)
    pos = np.arange(etot)
    j_within = pos - grp_start[grp_key]
    slot = goff[wloc] * 128 + j_within
    gixl_u[core_id, slot] = ssrc
    gixr_u[core_id, slot] = dl
    q_u[core_id, slot] = (dl - 128 * wloc).astype(np.float32)
    slot_of_edge[:] = slot

    def wrap16(u):  # [nslot] -> [128, nslot//16] int16, replicated over 8 q7 cores
        t = u.reshape(-1, 16).T.astype(np.int16)
        return np.tile(t, (8, 1))

    cores = []
    for c in range(ncores):
        cores.append(dict(
            gixl=wrap16(gixl_u[c]),
            gixr=wrap16(gixr_u[c]),
            qloc=np.ascontiguousarray(q_u[c].reshape(nchunk, 128).T),
        ))

    meta = dict(n=n, etot=etot, nloc=nloc, nw=nw, kw=[int(x) for x in kw],
                nchunk=nchunk, nslot=nslot, goff=[int(x) for x in goff],
                perm=perm, core_id=core_id, slot_of_edge=slot_of_edge, dl=dl)
    return cores, meta


# ---------------------------------------------------------------------------
# bass program
# ---------------------------------------------------------------------------

def build_nc(meta, nlayers=3, max_stage=4, use_gather=True, edge_cut=4):
    import concourse.bass as bass
    import concourse.bacc as bacc
    import concourse.mybir as mybir
    from concourse.tile import TileContext

    dt = mybir.dt
    f32 = dt.float32
    AF = mybir.ActivationFunctionType
    OP = mybir.AluOpType

    nloc, nw, kw, nchunk, nslot = (meta["nloc"], meta["nw"], meta["kw"],
                                   meta["nchunk"], meta["nslot"])
    goff = meta["goff"]
    n = meta["n"]
    nt = (nloc + 127) // 128                      # node tiles per core
    tsz = [min(128, nloc - 128 * t) for t in range(nt)]
    wsz = [min(128, nloc - 128 * w) for w in range(nw)]
    EW = HC + EX                                  # extended row: feats + lin terms + pad

    nc = bacc.Bacc(None, target_bir_lowering=False, debug=False)

    # ---- I/O ----
    def inp(name, shape, dtype=f32):
        return nc.declare_dram_parameter(name, list(shape), dtype, isOutput=False)

    def outp(name, shape, dtype=f32):
        return nc.declare_dram_parameter(name, list(shape), dtype, isOutput=True)

    xT4 = inp("xT4", [128, IN // 128, nloc])
    gixl = inp("gixl", [128, nslot // 16], dt.int16)
    gixr = inp("gixr", [128, nslot // 16], dt.int16)
    qloc = inp("qloc", [128, nchunk])
    iota = inp("iota", [128, 128])
    ident = inp("ident", [128, 128])
    onesc = inp("onesc", [128, 1])

    L = []
    for l in (1, 2, 3):
        L.append(dict(
            wlT=inp(f"wlT{l}", [128, 4, HC]),
            wrT=inp(f"wrT{l}", [128, 4, HC]),
            wal=inp(f"wal{l}", [128, 4, EX]),
            war=inp(f"war{l}", [128, 4, EX]),
            attbc=inp(f"attbc{l}", [128, HC]),
            blx=inp(f"blx{l}", [128, EW]),
            brx=inp(f"brx{l}", [128, EW]),
            gcm=inp(f"gcm{l}", [128, 4 if l < 3 else 1]),
            bcm=inp(f"bcm{l}", [128, 4 if l < 3 else 1]),
            ez=outp(f"ez{l}", [128, nchunk * 4]),
            rdn=outp(f"rdn{l}", [128, nw * 4]),
        ))
    z_out = outp("z_cm", [128, nloc])

    with TileContext(nc) as tc:
        with tc.tile_pool(name="const", bufs=1) as cpool, \
             tc.tile_pool(name="wts", bufs=1) as wpool, \
             tc.tile_pool(name="hcm", bufs=1) as hpool, \
             tc.tile_pool(name="big", bufs=1) as bpool, \
             tc.tile_pool(name="gath", bufs=2) as gpool, \
             tc.tile_pool(name="sm", bufs=3) as spool, \
             tc.tile_pool(name="scr", bufs=2) as scrpool, \
             tc.tile_pool(name="psv", bufs=2, space="PSUM") as psv, \
             tc.tile_pool(name="psagg", bufs=2, space="PSUM") as psagg, \
             tc.tile_pool(name="pssm", bufs=3, space="PSUM") as pssm, \
             tc.tile_pool(name="psst", bufs=1, space="PSUM") as psst, \
             tc.tile_pool(name="dram", bufs=1, space="DRAM") as dpool:

            # one pool register per distinct gather size (dma_gather would
            # otherwise allocate a fresh register per call and exhaust the pool)
            gsz_regs = {}
            for _k in set(kw):
                for _g in range(0, _k, G_CHUNKS):
                    _gsz = min(G_CHUNKS, _k - _g)
                    if _gsz not in gsz_regs:
                        r = nc.gpsimd.alloc_register(f"gsz{_gsz}")
                        nc.gpsimd.reg_mov(r, _gsz * 128)
                        gsz_regs[_gsz] = r

            # ---- persistent constants ----
            t_iota = cpool.tile([128, 128], f32, tag="iota")
            nc.sync.dma_start(t_iota[:], iota[:])
            t_I = cpool.tile([128, 128], f32, tag="ident")
            nc.sync.dma_start(t_I[:], ident[:])
            t_1 = cpool.tile([128, 1], f32, tag="onesc")
            nc.sync.dma_start(t_1[:], onesc[:])
            t_gixl = cpool.tile([128, nslot // 16], dt.int16, tag="gixl")
            nc.sync.dma_start(t_gixl[:], gixl[:])
            t_gixr = cpool.tile([128, nslot // 16], dt.int16, tag="gixr")
            nc.sync.dma_start(t_gixr[:], gixr[:])
            t_q = cpool.tile([128, nchunk], f32, tag="qloc")
            nc.sync.dma_start(t_q[:], qloc[:])

            # layer-1 node features (channel-major)
            h_cm = hpool.tile([128, 4, nloc], f32, tag="hcm")
            nc.sync.dma_start(h_cm[:], xT4[:])

            for li in range(nlayers):
                lay = L[li]
                last = li == 2
                cb = 1 if last else 4

                # ---- load layer weights ----
                t_wl = wpool.tile([128, 4, HC], f32, tag="wl")
                nc.sync.dma_start(t_wl[:], lay["wlT"][:])
                t_wr = wpool.tile([128, 4, HC], f32, tag="wr")
                nc.sync.dma_start(t_wr[:], lay["wrT"][:])
                t_wal = wpool.tile([128, 4, EX], f32, tag="wal")
                nc.sync.dma_start(t_wal[:], lay["wal"][:])
                t_war = wpool.tile([128, 4, EX], f32, tag="war")
                nc.sync.dma_start(t_war[:], lay["war"][:])
                t_att = wpool.tile([128, HC], f32, tag="att")
                nc.sync.dma_start(t_att[:], lay["attbc"][:])
                t_blx = wpool.tile([128, EW], f32, tag="blx")
                nc.sync.dma_start(t_blx[:], lay["blx"][:])
                t_brx = wpool.tile([128, EW], f32, tag="brx")
                nc.sync.dma_start(t_brx[:], lay["brx"][:])
                t_g = wpool.tile([128, cb], f32, tag="gcm")
                nc.sync.dma_start(t_g[:], lay["gcm"][:])
                t_b = wpool.tile([128, cb], f32, tag="bcm")
                nc.sync.dma_start(t_b[:], lay["bcm"][:])

                # ---- dram buffers ----
                d_xl = dpool.tile([nloc, EW], f32, tag="d_xl")
                d_xr = dpool.tile([nloc, EW], f32, tag="d_xr")
                d_xlfull = dpool.tile([n, EW], f32, tag="d_xlfull")
                d_sin = dpool.tile([128, 2 * cb], f32, tag="d_sin")
                d_sout = dpool.tile([128, 2 * cb], f32, tag="d_sout")

                # ---- projections of the local node slice ----
                for side, t_w, t_wa, t_bias, d_dst in (
                        ("l", t_wl, t_wal, t_blx, d_xl),
                        ("r", t_wr, t_war, t_brx, d_xr)):
                    for t in range(nt):
                        sz = tsz[t]
                        tsl = slice(128 * t, 128 * t + sz)
                        p_x = psv.tile([128, HC], f32, tag="psv")
                        p_s = pssm.tile([128, HC], f32, tag="pssm")
                        for kb in range(4):
                            nc.tensor.matmul(p_x[:sz], h_cm[:, kb, tsl],
                                             t_w[:, kb, :],
                                             start=(kb == 0), stop=(kb == 3))
                            nc.tensor.matmul(p_s[:sz, 0:EX], h_cm[:, kb, tsl],
                                             t_wa[:, kb, :],
                                             start=(kb == 0), stop=(kb == 3))
                        x_sb = scrpool.tile([128, EW], f32, tag="evac")
                        nc.vector.tensor_tensor(x_sb[:sz, 0:HC], p_x[:sz],
                                                t_bias[:sz, 0:HC], op=OP.add)
                        nc.vector.tensor_tensor(x_sb[:sz, HC:EW], p_s[:sz, 0:EX],
                                                t_bias[:sz, HC:EW], op=OP.add)
                        nc.sync.dma_start(d_dst[tsl, :], x_sb[:sz])

                if max_stage < 2:
                    continue
                # ---- allgather xl (node-major, gather-ready) ----
                nc.gpsimd.collective_compute(
                    "AllGather", mybir.AluOpType.bypass,
                    ins=[d_xl.opt()], outs=[d_xlfull.opt()],
                    replica_groups=[list(range(NCORES))])

                if max_stage < 3:
                    continue
                # ---- edge pass ----
                hpre = bpool.tile([128, nw, HC], f32, tag="hpre")
                stats_sb = spool.tile([128, 8], f32, tag="stats")
                rdn_w = spool.tile([128, nw, 4], f32, tag="rdnw")
                if last:
                    hm = bpool.tile([128, nw, C], f32, tag="hm")
                else:
                    hm = None
                for w in range(nw):
                    k = kw[w]
                    p_agg = psagg.tile([128, HC], f32, tag="psagg")
                    p_den = pssm.tile([128, HC], f32, tag="pssm")
                    ez_w = spool.tile([128, max(kw), 4], f32, tag="ezw")
                    t_xlg = t_xrg = None
                    for j in range(k):
                        gi = goff[w] + j
                        if j % G_CHUNKS == 0:
                            gsz = min(G_CHUNKS, k - j)
                            isl = slice(gi * 8, (gi + gsz) * 8)
                            t_xlg = gpool.tile([128, G_CHUNKS, EW], f32, tag="xlg")
                            t_xrg = gpool.tile([128, G_CHUNKS, EW], f32, tag="xrg")
                            if use_gather:
                                nc.gpsimd.dma_gather(
                                    t_xlg[:, 0:gsz, :], d_xlfull[:, :], t_gixl[:, isl],
                                    gsz * 128, gsz_regs[gsz], EW, elem_step=EW)
                                nc.gpsimd.dma_gather(
                                    t_xrg[:, 0:gsz, :], d_xr[:, :], t_gixr[:, isl],
                                    gsz * 128, gsz_regs[gsz], EW, elem_step=EW)
                            else:
                                for _c in range(gsz):
                                    _r0 = ((gi + _c) * 128) % max(1, nloc - 127)
                                    nc.sync.dma_start(
                                        t_xlg[:, _c, :], d_xr[_r0:_r0 + 128, :])
                                    nc.sync.dma_start(
                                        t_xrg[:, _c, :], d_xr[_r0:_r0 + 128, :])
                        s = j % G_CHUNKS
                        xl_t = t_xlg[:, s, :]
                        xr_t = t_xrg[:, s, :]

                        p_v = psv.tile([128, HC], f32, tag="psv")
                        p_lin = pssm.tile([128, HC], f32, tag="pssm")
                        nc.tensor.matmul(p_v[:], t_I[:], xl_t[:, 0:HC],
                                         start=True, stop=False)
                        nc.tensor.matmul(p_v[:], t_I[:], xr_t[:, 0:HC],
                                         start=False, stop=True)
                        nc.tensor.matmul(p_lin[:, 0:EX], t_I[:], xl_t[:, HC:EW],
                                         start=True, stop=False)
                        nc.tensor.matmul(p_lin[:, 0:EX], t_I[:], xr_t[:, HC:EW],
                                         start=False, stop=True)

                        rv = scrpool.tile([128, HC], f32, tag="rv")
                        nc.scalar.activation(rv[:], p_v[:], AF.Relu)
                        if edge_cut < 2:
                            nc.vector.tensor_copy(ez_w[:, j, :], p_lin[:, 0:4])
                            continue

                        lg = spool.tile([128, 4], f32, tag="lg")
                        for h in range(H):
                            hsl = slice(C * h, C * (h + 1))
                            scr = scrpool.tile([128, C], f32, tag="ttr_scr")
                            nc.vector.scalar_tensor_tensor(
                                scr[:], rv[:, hsl], 1.0, t_att[:, hsl],
                                op0=OP.mult, op1=OP.mult,
                                accum_out=lg[:, h:h + 1])
                        lg2 = spool.tile([128, 4], f32, tag="lg2")
                        nc.vector.scalar_tensor_tensor(
                            lg2[:], lg[:], 1.0 - ATT_NEG, p_lin[:, 0:H],
                            op0=OP.mult, op1=OP.add)
                        nc.scalar.activation(ez_w[:, j, :], lg2[:], AF.Exp)

                        if edge_cut < 3:
                            continue
                        for h in range(H):
                            hsl = slice(C * h, C * (h + 1))
                            mez = scrpool.tile([128, 128], f32, tag="mez")
                            nc.vector.tensor_scalar(
                                mez[:], t_iota[:], t_q[:, gi:gi + 1],
                                ez_w[:, j, h:h + 1],
                                op0=OP.is_equal, op1=OP.mult)
                            nc.tensor.matmul(p_agg[:, hsl], mez[:], xl_t[:, hsl],
                                             start=(j == 0 and h == 0),
                                             stop=(j == k - 1 and h == H - 1))
                            nc.tensor.matmul(p_den[:, h:h + 1], mez[:], t_1[:],
                                             start=(j == 0 and h == 0),
                                             stop=(j == k - 1 and h == H - 1))

                    # ---- window epilogue ----
                    if edge_cut < 3:
                        continue
                    nc.sync.dma_start(
                        lay["ez"][:, goff[w] * 4:(goff[w] + k) * 4],
                        ez_w[:, 0:k, :])
                    den_e = spool.tile([128, 4], f32, tag="dene")
                    nc.vector.tensor_scalar(den_e[:], p_den[:, 0:4], 1e-30, None,
                                            op0=OP.add)
                    nc.vector.reciprocal(rdn_w[:, w, :], den_e[:])
                    if w == nw - 1:
                        nc.sync.dma_start(lay["rdn"][:, :], rdn_w[:])
                    for h in range(H):
                        hsl = slice(C * h, C * (h + 1))
                        nc.vector.tensor_scalar(
                            hpre[:, w, hsl], p_agg[:, hsl],
                            rdn_w[:, w, h:h + 1], None, op0=OP.mult)

                    # feat used for stats / output
                    if last:
                        ha = scrpool.tile([128, C], f32, tag="ha")
                        hb = scrpool.tile([128, C], f32, tag="hb")
                        nc.vector.tensor_tensor(ha[:], hpre[:, w, 0:C],
                                                hpre[:, w, C:2 * C], op=OP.add)
                        nc.vector.tensor_tensor(hb[:], hpre[:, w, 2 * C:3 * C],
                                                hpre[:, w, 3 * C:4 * C], op=OP.add)
                        nc.vector.tensor_tensor(hb[:], ha[:], hb[:], op=OP.add)
                        nc.vector.tensor_scalar(hm[:, w, :], hb[:], 0.25, None,
                                                op0=OP.mult)
                        feat = hm
                    else:
                        feat = hpre

                    if edge_cut < 4:
                        continue
                    p_st = psst.tile([128, HC], f32, tag="psst")
                    sq = scrpool.tile([128, HC], f32, tag="sq")
                    nfeat = C * cb
                    nc.scalar.activation(sq[:, 0:nfeat], feat[:, w, 0:nfeat],
                                         AF.Square)
                    for b in range(cb):
                        bsl = slice(C * b, C * (b + 1))
                        nc.tensor.matmul(p_st[:, b:b + 1], feat[:, w, bsl],
                                         t_1[:], start=(b == 0), stop=False)
                        nc.tensor.matmul(p_st[:, cb + b:cb + b + 1], sq[:, bsl],
                                         t_1[:], start=False, stop=(b == cb - 1))
                    if w == 0:
                        nc.vector.tensor_copy(stats_sb[:, 0:2 * cb],
                                              p_st[:, 0:2 * cb])
                    else:
                        nc.vector.tensor_tensor(stats_sb[:, 0:2 * cb],
                                                stats_sb[:, 0:2 * cb],
                                                p_st[:, 0:2 * cb], op=OP.add)

                if max_stage < 4:
                    continue
                # ---- batchnorm stats allreduce ----
                nc.sync.dma_start(d_sin[:, 0:2 * cb], stats_sb[:, 0:2 * cb])
                nc.gpsimd.collective_compute(
                    "AllReduce", mybir.AluOpType.add,
                    ins=[d_sin.opt()], outs=[d_sout.opt()],
                    replica_groups=[list(range(NCORES))])
                gstats = spool.tile([128, 8], f32, tag="gstats")
                nc.sync.dma_start(gstats[:, 0:2 * cb], d_sout[:, :])

                mu = spool.tile([128, 4], f32, tag="mu")
                nc.vector.tensor_scalar(mu[:, 0:cb], gstats[:, 0:cb], 1.0 / n,
                                        None, op0=OP.mult)
                var = spool.tile([128, 4], f32, tag="var")
                # var = ex2 - mu^2 ; compute ex2 into var first
                nc.vector.tensor_scalar(var[:, 0:cb], gstats[:, cb:2 * cb],
                                        1.0 / n, None, op0=OP.mult)
                mu2 = spool.tile([128, 4], f32, tag="mu2")
                nc.vector.tensor_tensor(mu2[:, 0:cb], mu[:, 0:cb], mu[:, 0:cb],
                                        op=OP.mult)
                nc.vector.tensor_tensor(var[:, 0:cb], var[:, 0:cb],
                                        mu2[:, 0:cb], op=OP.subtract)
                sd = spool.tile([128, 4], f32, tag="sd")
                nc.vector.tensor_scalar(sd[:, 0:cb], var[:, 0:cb], 1e-5, None,
                                        op0=OP.add)
                nc.scalar.activation(sd[:, 0:cb], sd[:, 0:cb], AF.Sqrt)
                rsd = spool.tile([128, 4], f32, tag="rsd")
                nc.vector.reciprocal(rsd[:, 0:cb], sd[:, 0:cb])
                s_ap = spool.tile([128, 4], f32, tag="s_ap")
                nc.vector.tensor_tensor(s_ap[:, 0:cb], t_g[:], rsd[:, 0:cb],
                                        op=OP.mult)
                t_ap = spool.tile([128, 4], f32, tag="t_ap")
                nc.vector.tensor_tensor(t_ap[:, 0:cb], mu[:, 0:cb], s_ap[:, 0:cb],
                                        op=OP.mult)
                nc.vector.tensor_tensor(t_ap[:, 0:cb], t_b[:], t_ap[:, 0:cb],
                                        op=OP.subtract)
                s01 = spool.tile([128, 4], f32, tag="s01")
                nc.vector.tensor_scalar(s01[:, 0:cb], s_ap[:, 0:cb], NEG, None,
                                        op0=OP.mult)
                t01 = spool.tile([128, 4], f32, tag="t01")
                nc.vector.tensor_scalar(t01[:, 0:cb], t_ap[:, 0:cb], NEG, None,
                                        op0=OP.mult)

                # ---- bn apply + leaky relu + transpose to channel-major ----
                if last:
                    z_sb = bpool.tile([128, nloc], f32, tag="znext")
                else:
                    h_next = hpool.tile([128, 4, nloc], f32, tag="hcm")
                feat = hm if last else hpre
                for w in range(nw):
                    sz = wsz[w]
                    p_t = psv.tile([128, HC], f32, tag="psv")
                    for b in range(cb):
                        bsl = slice(C * b, C * (b + 1))
                        nc.tensor.matmul(p_t[:, bsl], feat[:, w, bsl], t_I[:],
                                         is_transpose=True,
                                         start=(b == 0), stop=(b == cb - 1))
                    for b in range(cb):
                        bsl = slice(C * b, C * b + sz)
                        r1 = scrpool.tile([128, 128], f32, tag="r1")
                        nc.scalar.activation(r1[:, 0:sz], p_t[:, bsl], AF.Relu,
                                             scale=s_ap[:, b:b + 1],
                                             bias=t_ap[:, b:b + 1])
                        b01 = scrpool.tile([128, 128], f32, tag="b01")
                        nc.scalar.activation(b01[:, 0:sz], p_t[:, bsl],
                                             AF.Identity,
                                             scale=s01[:, b:b + 1],
                                             bias=t01[:, b:b + 1])
                        osl = slice(128 * w, 128 * w + sz)
                        if last:
                            nc.vector.scalar_tensor_tensor(
                                z_sb[:, osl], r1[:, 0:sz], 1.0 - NEG,
                                b01[:, 0:sz], op0=OP.mult, op1=OP.add)
                        else:
                            nc.vector.scalar_tensor_tensor(
                                h_next[:, b, osl], r1[:, 0:sz], 1.0 - NEG,
                                b01[:, 0:sz], op0=OP.mult, op1=OP.add)
                if last:
                    nc.sync.dma_start(z_out[:, :], z_sb[:])
                else:
                    if li == nlayers - 1:  # truncated debug build
                        nc.sync.dma_start(z_out[:, :], h_next[:, 0, :])
                    h_cm = h_next

    nc.finalize()
    return nc


# ---------------------------------------------------------------------------
# weight / input packing
# ---------------------------------------------------------------------------

def pack_shared_inputs(params, meta):
    """params: dict with reference weight arrays. Returns name->np.ndarray."""
    out = {}
    out["iota"] = np.tile(np.arange(128, dtype=np.float32)[None, :], (128, 1))
    out["ident"] = np.eye(128, dtype=np.float32)
    out["onesc"] = np.ones((128, 1), np.float32)

    for l in (1, 2, 3):
        wl = params[f"w{l}_l"].astype(np.float32)   # [HC, IN]
        wr = params[f"w{l}_r"].astype(np.float32)
        bl = params[f"b{l}_l"].astype(np.float32)   # [HC]
        br = params[f"b{l}_r"].astype(np.float32)
        att = params[f"att{l}"].astype(np.float32)  # [H, C]
        gcm = params[f"bn{l}_g"].astype(np.float32)
        bcm = params[f"bn{l}_b"].astype(np.float32)

        attW = np.zeros((HC, H), np.float32)
        for h in range(H):
            attW[C * h:C * (h + 1), h] = att[h]

        wlT = np.ascontiguousarray(wl.T)            # [IN, HC]
        wrT = np.ascontiguousarray(wr.T)
        out[f"wlT{l}"] = wlT.reshape(4, 128, HC).transpose(1, 0, 2).copy()
        out[f"wrT{l}"] = wrT.reshape(4, 128, HC).transpose(1, 0, 2).copy()
        wal = np.zeros((IN, EX), np.float32)
        wal[:, 0:H] = (wlT @ attW) * ATT_NEG
        war = np.zeros((IN, EX), np.float32)
        war[:, 0:H] = (wrT @ attW) * ATT_NEG
        out[f"wal{l}"] = wal.reshape(4, 128, EX).transpose(1, 0, 2).copy()
        out[f"war{l}"] = war.reshape(4, 128, EX).transpose(1, 0, 2).copy()
        out[f"attbc{l}"] = np.tile(att.reshape(1, HC), (128, 1))
        pad = np.zeros(EX - H, np.float32)
        blx = np.concatenate([bl, (bl @ attW) * ATT_NEG, pad])
        brx = np.concatenate([br, (br @ attW) * ATT_NEG, pad])
        out[f"blx{l}"] = np.tile(blx[None, :], (128, 1))
        out[f"brx{l}"] = np.tile(brx[None, :], (128, 1))
        if l < 3:
            out[f"gcm{l}"] = np.ascontiguousarray(gcm.reshape(4, 128).T)
            out[f"bcm{l}"] = np.ascontiguousarray(bcm.reshape(4, 128).T)
        else:
            out[f"gcm{l}"] = gcm.reshape(128, 1).copy()
            out[f"bcm{l}"] = bcm.reshape(128, 1).copy()
    return out


def pack_core_inputs(x, core_arrays, meta, c):
    nloc = meta["nloc"]
    xloc = x[c * nloc:(c + 1) * nloc].astype(np.float32)      # [nloc, IN]
    xT = xloc.T.reshape(4, 128, nloc).transpose(1, 0, 2).copy()
    d = dict(core_arrays[c])
    d["xT4"] = xT
    return d


def postprocess(results, meta):
    nloc, nw, nchunk = meta["nloc"], meta["nw"], meta["nchunk"]
    etot = meta["etot"]
    perm, core_id, slot, dl = (meta["perm"], meta["core_id"],
                               meta["slot_of_edge"], meta["dl"])
    z = np.concatenate([results[c]["z_cm"].T for c in range(NCORES)], axis=0)
    alphas = []
    for l in (1, 2, 3):
        a_sorted = np.empty((etot, H), np.float32)
        for c in range(NCORES):
            sel = core_id == c
            sl = slot[sel]
            dlc = dl[sel]
            ez = results[c][f"ez{l}"].reshape(128, nchunk, 4)
            rdn = results[c][f"rdn{l}"].reshape(128, nw, 4)
            a_sorted[sel] = (ez[sl % 128, sl // 128, :] *
                             rdn[dlc % 128, dlc // 128, :])
        a = np.empty((etot, H), np.float32)
        a[perm] = a_sorted
        alphas.append(a)
    gate = np.ones((meta["n"], 1), np.float32)
    return z, alphas[0], alphas[1], alphas[2], gate


# ---------------------------------------------------------------------------
# entry point
# ---------------------------------------------------------------------------

def kernel(**inputs):
    from concourse.bass_utils import run_bass_kernel_spmd

    x = np.asarray(inputs["x"], np.float32)
    edge_index = np.asarray(inputs["edge_index"])
    core_arrays, meta = preprocess(edge_index, N, NCORES)
    nc = build_nc(meta)
    shared = pack_shared_inputs(inputs, meta)
    in_maps = []
    for c in range(NCORES):
        m = dict(shared)
        m.update(pack_core_inputs(x, core_arrays, meta, c))
        in_maps.append(m)
    res = run_bass_kernel_spmd(nc, in_maps, list(range(NCORES)))
    return postprocess(res.results, meta)
